# revision 9
# baseline (speedup 1.0000x reference)
"""DualAdaptiveQuantizer TRN2 kernel.

Data-parallel over B (32 -> 4 per core x 8 cores). Per core:
  phase 1: h_power = sum_{n,c} H^2        (ACT square + DVE reduce, PE row-sum for mean)
  phase 2: tiny router MLP on PE (block-diag packed x4, fp32), logits -> DRAM round-trip
  phase 3: argmax one-hot + per-group quant constants (DVE/ACT/GPSIMD)
  phase 4: H fake-quant: s*round(clip(H/s)) via RNE add-magic trick (DVE)
  phase 5: v fake-quant (DVE)
"""
import numpy as np
from contextlib import ExitStack

import concourse.bacc as bacc
import concourse.mybir as mybir
import concourse.tile as tile
from concourse.bass_utils import run_bass_kernel_spmd

F32 = mybir.dt.float32
AF = mybir.ActivationFunctionType
OP = mybir.AluOpType
RNE_C = 12582912.0  # 1.5 * 2**23
DEBUG_SCRATCH = False

NCORES = 8
B, L, K, N = 32, 1024, 64, 8
BB = B // NCORES          # batches per core
T = BB * L * K            # tokens (groups) per core
NT = 8                    # row-tiles per batch (L/128)
NQ = 4                    # MLP token-block packing factor
TOKB = T // NQ            # tokens per block (65536)
NSB = 4                   # pinT sub-batches per batch
SBW = L * K // NQ // NSB  # pinT columns per sub-batch (4096)
NCH = L * K // NQ // 512  # 512-col chunks per batch (32)


def _build(s2d, s4d, s2c, s4c):
    nc = bacc.Bacc("TRN2", target_bir_lowering=False)

    v = nc.dram_tensor("v", [BB, L, K, 2], F32, kind="ExternalInput")
    H = nc.dram_tensor("H", [BB, L, N, K, 2], F32, kind="ExternalInput")
    snr = nc.dram_tensor("snr", [BB, L, K], F32, kind="ExternalInput")
    g_d = nc.dram_tensor("g_d", [BB, L, K, 3], F32, kind="ExternalInput")
    g_c = nc.dram_tensor("g_c", [BB, L, K, 3], F32, kind="ExternalInput")
    W1bd = nc.dram_tensor("W1bd", [5 * NQ, 32 * NQ], F32, kind="ExternalInput")
    W2bd = nc.dram_tensor("W2bd", [32 * NQ, 32 * NQ], F32, kind="ExternalInput")
    Wdcbd = nc.dram_tensor("Wdcbd", [32 * NQ, 6 * NQ], F32, kind="ExternalInput")
    b1p = nc.dram_tensor("b1p", [32 * NQ, 1], F32, kind="ExternalInput")
    b2p = nc.dram_tensor("b2p", [32 * NQ, 1], F32, kind="ExternalInput")

    v_q = nc.dram_tensor("v_q", [BB, L, K, 2], F32, kind="ExternalOutput")
    H_q = nc.dram_tensor("H_q", [BB, L, N, K, 2], F32, kind="ExternalOutput")
    e_d = nc.dram_tensor("e_d", [BB, L, K], F32, kind="ExternalOutput")
    e_c = nc.dram_tensor("e_c", [BB, L, K], F32, kind="ExternalOutput")
    w_d = nc.dram_tensor("w_d", [BB, L, K, 3], F32, kind="ExternalOutput")
    w_c = nc.dram_tensor("w_c", [BB, L, K, 3], F32, kind="ExternalOutput")

    dbg = "ExternalOutput" if DEBUG_SCRATCH else "Internal"
    hp_s = nc.dram_tensor("hp_s", [BB, L, K], F32, kind=dbg)
    avg_s = nc.dram_tensor("avg_s", [BB, K], F32, kind=dbg)
    LgT = nc.dram_tensor("LgT", [6 * NQ, BB * (L // NQ) * K], F32, kind=dbg)

    r2d, r4d = 1.0 / np.float32(s2d), 1.0 / np.float32(s4d)
    r2c, r4c = 1.0 / np.float32(s2c), 1.0 / np.float32(s4c)

    # DRAM views
    H_tiles = H[:].rearrange("b (t r) n k c -> b t r (n k c)", t=NT, r=128)
    Hq_tiles = H_q[:].rearrange("b (t r) n k c -> b t r (n k c)", t=NT, r=128)
    v_tiles = v[:].rearrange("b (t r) k c -> b t r (k c)", t=NT, r=128)
    vq_tiles = v_q[:].rearrange("b (t r) k c -> b t r (k c)", t=NT, r=128)
    hp_tiles = hp_s[:].rearrange("b (t r) k -> b t r k", t=NT, r=128)

    gd_view = g_d[:].rearrange("b (t r) k cls -> b r t (k cls)", t=NT, r=128)
    gc_view = g_c[:].rearrange("b (t r) k cls -> b r t (k cls)", t=NT, r=128)
    ed_view = e_d[:].rearrange("b (t r) k -> b r t k", t=NT, r=128)
    ec_view = e_c[:].rearrange("b (t r) k -> b r t k", t=NT, r=128)
    wd_view = w_d[:].rearrange("b (t r) k cls -> b r t (k cls)", t=NT, r=128)
    wc_view = w_c[:].rearrange("b (t r) k cls -> b r t (k cls)", t=NT, r=128)
    v_feat = v[:].rearrange("b (q m) k c -> b q c (m k)", q=NQ, m=L // NQ)
    snr_feat = snr[:].rearrange("b (q m) k -> b q (m k)", q=NQ, m=L // NQ)
    hp_feat = hp_s[:].rearrange("b (q m) k -> b q (m k)", q=NQ, m=L // NQ)

    with tile.TileContext(nc) as tc, ExitStack() as ctx:
        wpool = ctx.enter_context(tc.tile_pool(name="wp", bufs=1))
        pool = ctx.enter_context(tc.tile_pool(name="p", bufs=1))
        psum = ctx.enter_context(tc.tile_pool(name="ps", bufs=2, space="PSUM"))

        W1t = wpool.tile([5 * NQ, 32 * NQ], F32, tag="W1t")
        nc.sync.dma_start(W1t[:], W1bd[:])
        W2t = wpool.tile([32 * NQ, 32 * NQ], F32, tag="W2t")
        nc.sync.dma_start(W2t[:], W2bd[:])
        Wdct = wpool.tile([32 * NQ, 6 * NQ], F32, tag="Wdct")
        nc.sync.dma_start(Wdct[:], Wdcbd[:])
        b1t = wpool.tile([32 * NQ, 1], F32, tag="b1t")
        nc.sync.dma_start(b1t[:], b1p[:])
        b2t = wpool.tile([32 * NQ, 1], F32, tag="b2t")
        nc.sync.dma_start(b2t[:], b2p[:])
        ones = wpool.tile([128, 1], F32, tag="ones")
        nc.vector.memset(ones[:], 1.0)

        for b in range(BB):
            # ---------------- phase 1: h_power ----------------
            hps = psum.tile([K, 1], F32, tag="hps")
            for t in range(NT):
                Ht = pool.tile([128, 1024], F32, tag="Ht", bufs=3)
                nc.sync.dma_start(Ht[:], H_tiles[b, t])
                sq = pool.tile([128, 1024], F32, tag="sq", bufs=2)
                nc.scalar.activation(sq[:], Ht[:], AF.Square)
                hp = pool.tile([128, K], F32, tag="hp", bufs=2)
                nc.vector.tensor_reduce(
                    hp[:], sq[:].rearrange("p (n k c) -> p k n c", n=N, k=K, c=2),
                    axis=mybir.AxisListType.XY, op=OP.add)
                nc.sync.dma_start(hp_tiles[b, t], hp[:])
                nc.tensor.matmul(hps[:], hp[:], ones[:], start=(t == 0), stop=(t == NT - 1))
            avgt = pool.tile([K, 1], F32, tag="avgt", bufs=2)
            nc.scalar.activation(avgt[:], hps[:], AF.Copy, scale=1.0 / L)
            nc.sync.dma_start(avg_s[b].unsqueeze(1), avgt[:])

            # ---------------- phase 2: MLP ----------------
            for sb in range(NSB):
                pinT = pool.tile([5 * NQ, SBW], F32, tag="pinT", bufs=2)
                for q in range(NQ):
                    sl = slice(sb * SBW, (sb + 1) * SBW)
                    r0 = 5 * q
                    nc.sync.dma_start(pinT[r0:r0 + 2], v_feat[b, q][:, sl])
                    nc.sync.dma_start(pinT[r0 + 2:r0 + 3], snr_feat[b, q, sl].unsqueeze(0))
                    nc.sync.dma_start(pinT[r0 + 3:r0 + 4], hp_feat[b, q, sl].unsqueeze(0))
                    nc.sync.dma_start(
                        pinT[r0 + 4:r0 + 5].rearrange("p (a k) -> p a k", k=K),
                        avg_s[b].unsqueeze(0).unsqueeze(1).broadcast_to([1, SBW // K, K]))
                for cc in range(SBW // 512):
                    c = sb * (SBW // 512) + cc
                    csl = slice(cc * 512, (cc + 1) * 512)
                    ps1 = psum.tile([128, 512], F32, tag="ps1")
                    nc.tensor.matmul(ps1[:], W1t[:], pinT[:, csl], start=True, stop=True)
                    h1 = pool.tile([128, 512], F32, tag="h1", bufs=2)
                    nc.scalar.activation(h1[:], ps1[:], AF.Relu, bias=b1t[:])
                    ps2 = psum.tile([128, 512], F32, tag="ps2")
                    nc.tensor.matmul(ps2[:], W2t[:], h1[:], start=True, stop=True)
                    h2 = pool.tile([128, 512], F32, tag="h2", bufs=2)
                    nc.scalar.activation(h2[:], ps2[:], AF.Relu, bias=b2t[:])
                    ps3 = psum.tile([6 * NQ, 512], F32, tag="ps3")
                    nc.tensor.matmul(ps3[:], Wdct[:], h2[:], start=True, stop=True)
                    lg = pool.tile([6 * NQ, 512], F32, tag="lg", bufs=2)
                    nc.scalar.activation(lg[:], ps3[:], AF.Copy)
                    nc.sync.dma_start(
                        LgT[:, b * 16384 + c * 512: b * 16384 + (c + 1) * 512], lg[:])

            # ---------------- phase 3: decisions ----------------
            def decide(cls_off, G_src, bdc_off, beta_scale, e_view_b, w_view_b, wtag):
                Ld = pool.tile([128, 3 * 512], F32, tag="Ld")
                for cls in range(3):
                    for q in range(NQ):
                        src = LgT[cls_off + q * 6 + cls].rearrange(
                            "(bb tl r k) -> bb r tl k", bb=BB, tl=2, r=128, k=K)[b]
                        dst = Ld[:, cls * 512 + q * 128: cls * 512 + (q + 1) * 128]
                        nc.sync.dma_start(dst.rearrange("p (t k) -> p t k", t=2, k=K), src)
                G = pool.tile([128, 3 * 512], F32, tag="G")
                nc.sync.dma_start(G[:].rearrange("p (t kc) -> p t kc", t=NT), G_src)
                Gv = G[:].rearrange("p (t k cls) -> p cls (t k)", t=NT, k=K, cls=3)
                s_ = []
                for cls in range(3):
                    st = pool.tile([128, 512], F32, tag=f"s{cls}")
                    nc.vector.scalar_tensor_tensor(
                        st[:], Ld[:, cls * 512:(cls + 1) * 512], float(bdc_off[cls]),
                        Gv[:, cls], op0=OP.add, op1=OP.add)
                    s_.append(st)
                m = pool.tile([128, 512], F32, tag="m")
                nc.vector.tensor_tensor(m[:], s_[0][:], s_[1][:], OP.max)
                nc.vector.tensor_tensor(m[:], m[:], s_[2][:], OP.max)
                w0 = pool.tile([128, 512], F32, tag="w0")
                nc.vector.tensor_tensor(w0[:], s_[0][:], m[:], OP.is_equal)
                e1 = pool.tile([128, 512], F32, tag="e1")
                nc.vector.tensor_tensor(e1[:], s_[1][:], m[:], OP.is_equal)
                t1 = pool.tile([128, 512], F32, tag="t1")
                nc.scalar.activation(t1[:], w0[:], AF.Copy, scale=-1.0, bias=1.0)
                w1 = pool.tile([128, 512], F32, tag="w1", bufs=2)
                nc.vector.tensor_tensor(w1[:], e1[:], t1[:], OP.mult)
                w2 = pool.tile([128, 512], F32, tag="w2", bufs=2)
                nc.vector.tensor_tensor(w2[:], t1[:], w1[:], OP.subtract)
                u = pool.tile([128, 512], F32, tag="u")
                nc.vector.scalar_tensor_tensor(u[:], w2[:], 2.0, w1[:], op0=OP.mult, op1=OP.add)
                eb = pool.tile([128, 512], F32, tag="eb")
                nc.scalar.activation(eb[:], u[:], AF.Copy, scale=float(beta_scale))
                nc.sync.dma_start(e_view_b, eb[:].rearrange("p (t k) -> p t k", t=NT))
                wout = pool.tile([128, 3 * 512], F32, tag=wtag)
                wv = wout[:].rearrange("p (t k cls) -> p cls (t k)", t=NT, k=K, cls=3)
                nc.gpsimd.tensor_copy(wv[:, 0], w0[:])
                nc.gpsimd.tensor_copy(wv[:, 1], w1[:])
                nc.gpsimd.tensor_copy(wv[:, 2], w2[:])
                nc.sync.dma_start(w_view_b, wout[:].rearrange("p (t kc) -> p t kc", t=NT))
                return w1, w2

            w1d, w2d = decide(0, gd_view[b], bdc_d_vals,
                              4.0, ed_view[b], wd_view[b], "woutd")
            w1c, w2c = decide(3, gc_view[b], bdc_c_vals,
                              2.0 * N * 2, ec_view[b], wc_view[b], "woutc")

            # ------------- per-group quant constants (k-width then expand to kc) -------------
            def consts(w1, w2, r2, r4, s2, s4, pre):
                ta = pool.tile([128, 512], F32, tag="tmpk", bufs=2)
                nc.vector.tensor_scalar(ta[:], w1[:], float(r2 - 1.0), 1.0, op0=OP.mult, op1=OP.add)
                rsg_k = pool.tile([128, 512], F32, tag="constk", bufs=2)
                nc.vector.scalar_tensor_tensor(rsg_k[:], w2[:], float(r4 - 1.0), ta[:], op0=OP.mult, op1=OP.add)
                tb = pool.tile([128, 512], F32, tag="tmpk", bufs=2)
                nc.vector.tensor_scalar(tb[:], w1[:], -2.0, RNE_C, op0=OP.mult, op1=OP.add)
                qnC_k = pool.tile([128, 512], F32, tag="constk", bufs=2)
                nc.vector.scalar_tensor_tensor(qnC_k[:], w2[:], -8.0, tb[:], op0=OP.mult, op1=OP.add)
                qp_k = pool.tile([128, 512], F32, tag="constk", bufs=2)
                nc.vector.scalar_tensor_tensor(qp_k[:], w2[:], 7.0, w1[:], op0=OP.mult, op1=OP.add)
                tcm = pool.tile([128, 512], F32, tag="tmpk", bufs=2)
                nc.vector.tensor_scalar_mul(tcm[:], w1[:], float(s2))
                ms_k = pool.tile([128, 512], F32, tag="constk", bufs=2)
                nc.vector.scalar_tensor_tensor(ms_k[:], w2[:], float(s4), tcm[:], op0=OP.mult, op1=OP.add)
                out = []
                for kt, tagn in ((rsg_k, "rsg"), (qnC_k, "qnc"), (qp_k, "qp"), (ms_k, "ms")):
                    kc = pool.tile([128, 1024], F32, tag=f"{pre}{tagn}kc", bufs=1)
                    nc.gpsimd.tensor_copy(
                        kc[:].rearrange("p (tk c) -> p tk c", c=2),
                        kt[:].unsqueeze(2).broadcast_to([128, 512, 2]))
                    out.append(kc)
                return out

            rsg_c, qnC_c, qp_c, ms_c = consts(w1c, w2c, r2c, r4c, s2c, s4c, "c")
            rsg_d, qnC_d, qp_d, ms_d = consts(w1d, w2d, r2d, r4d, s2d, s4d, "d")

            # ---------------- phase 4: H fake-quant ----------------
            for t in range(NT):
                Ht = pool.tile([128, 1024], F32, tag="Ht", bufs=3)
                nc.sync.dma_start(Ht[:], H_tiles[b, t])
                ksl = slice(t * 128, (t + 1) * 128)

                def bcH(c_):
                    return c_[:, ksl].unsqueeze(1).broadcast_to([128, N, 128])
                y = pool.tile([128, 1024], F32, tag="yb", bufs=2)
                yv = y[:].rearrange("p (n m) -> p n m", n=N)
                z = pool.tile([128, 1024], F32, tag="zb", bufs=2)
                zv = z[:].rearrange("p (n m) -> p n m", n=N)
                nc.vector.tensor_tensor(yv, Ht[:].rearrange("p (n m) -> p n m", n=N), bcH(rsg_c), OP.mult)
                nc.vector.scalar_tensor_tensor(zv, yv, RNE_C, bcH(qnC_c), op0=OP.add, op1=OP.max)
                nc.vector.scalar_tensor_tensor(yv, zv, RNE_C, bcH(qp_c), op0=OP.subtract, op1=OP.min)
                nc.vector.tensor_tensor(zv, yv, bcH(ms_c), OP.mult)
                nc.sync.dma_start(Hq_tiles[b, t], z[:])

            # ---------------- phase 5: v fake-quant ----------------
            for t in range(NT):
                vt = pool.tile([128, 128], F32, tag="vt", bufs=2)
                nc.sync.dma_start(vt[:], v_tiles[b, t])
                ksl = slice(t * 128, (t + 1) * 128)
                yv2 = pool.tile([128, 128], F32, tag="yv", bufs=2)
                zv2 = pool.tile([128, 128], F32, tag="zv", bufs=2)
                nc.vector.tensor_tensor(yv2[:], vt[:], rsg_d[:, ksl], OP.mult)
                nc.vector.scalar_tensor_tensor(zv2[:], yv2[:], RNE_C, qnC_d[:, ksl], op0=OP.add, op1=OP.max)
                nc.vector.scalar_tensor_tensor(yv2[:], zv2[:], RNE_C, qp_d[:, ksl], op0=OP.subtract, op1=OP.min)
                nc.vector.tensor_tensor(zv2[:], yv2[:], ms_d[:, ksl], OP.mult)
                nc.sync.dma_start(vq_tiles[b, t], zv2[:])

    nc.compile()
    return nc


_CACHE = {}
LAST_RES = None
bdc_d_vals = None
bdc_c_vals = None


def kernel(v, H, local_snr, g_demod, g_channel,
           W1, b1, W2, b2, Wd, bd, Wc, bc, s2d, s4d, s2c, s4c):
    global bdc_d_vals, bdc_c_vals
    v = np.asarray(v, np.float32)
    H = np.asarray(H, np.float32)
    snr = np.asarray(local_snr, np.float32).reshape(B, L, K)
    g_d = np.asarray(g_demod, np.float32)
    g_c = np.asarray(g_channel, np.float32)
    s2d, s4d = float(s2d), float(s4d)
    s2c, s4c = float(s2c), float(s4c)

    # packed block-diagonal weights (4 token blocks)
    W1 = np.asarray(W1, np.float32); W2 = np.asarray(W2, np.float32)
    Wd = np.asarray(Wd, np.float32); Wc = np.asarray(Wc, np.float32)
    Wdc = np.concatenate([Wd, Wc], axis=1)                      # (32, 6)
    W1bd = np.zeros((5 * NQ, 32 * NQ), np.float32)
    W2bd = np.zeros((32 * NQ, 32 * NQ), np.float32)
    Wdcbd = np.zeros((32 * NQ, 6 * NQ), np.float32)
    for q in range(NQ):
        W1bd[5 * q:5 * q + 5, 32 * q:32 * q + 32] = W1
        W2bd[32 * q:32 * q + 32, 32 * q:32 * q + 32] = W2
        Wdcbd[32 * q:32 * q + 32, 6 * q:6 * q + 6] = Wdc
    b1p = np.tile(np.asarray(b1, np.float32), NQ)[:, None]
    b2p = np.tile(np.asarray(b2, np.float32), NQ)[:, None]
    bdc_d_vals = tuple(float(x) for x in np.asarray(bd, np.float32))
    bdc_c_vals = tuple(float(x) for x in np.asarray(bc, np.float32))

    key = (s2d, s4d, s2c, s4c, bdc_d_vals, bdc_c_vals)
    if key not in _CACHE:
        _CACHE[key] = _build(s2d, s4d, s2c, s4c)
    nc = _CACHE[key]

    in_maps = []
    for c in range(NCORES):
        sl = slice(c * BB, (c + 1) * BB)
        in_maps.append(dict(
            v=v[sl], H=H[sl], snr=snr[sl], g_d=g_d[sl], g_c=g_c[sl],
            W1bd=W1bd, W2bd=W2bd, Wdcbd=Wdcbd, b1p=b1p, b2p=b2p))
    res = run_bass_kernel_spmd(nc, in_maps, list(range(NCORES))).results
    global LAST_RES
    LAST_RES = res

    cat = lambda name: np.concatenate([r[name] for r in res], axis=0)
    return (cat("v_q"), cat("H_q"), cat("e_d"), cat("e_c"), cat("w_d"), cat("w_c"))
